# revision 1
# baseline (speedup 1.0000x reference)
"""GroupWiseLinear Trainium2 kernel.

out[b, c] = dot(W[0, c, :], x[b, group_of[c], :]) + bias[0, c], then a final
class-permutation gather, for two independent branches (co / cl).

Sharding: 8 cores = 2 branches x 4 class-quarters (1024 classes each, all 64
batches per core).  The ragged group segments of each core's class range are
split/padded on host into uniform 64-column "slots" so that every core runs
the SAME instruction stream (SPMD) on different data:

  - xt:  [128, S*4*64]  per-slot x^T (H-major), replicated per slot
  - wt:  [128, 4*S*64]  W^T (H-major), zero-padded to slot layout
  - bz:  [1, S*64]      bias, zero-padded to slot layout
  - out: [64, S*64]     padded per-core output (batch-major)

Device work per slot: 4 K-chunk matmuls (x stationary [128,64], W moving) that
accumulate into PSUM, plus a rank-1 ones-matmul adding the bias.  Host
"unshard" places each core's real columns into the final permuted output.
"""

import ml_dtypes
import numpy as np

import concourse.bacc as bacc
import concourse.tile as tile
from concourse import mybir
from concourse.bass_utils import run_bass_kernel_spmd

B = 64          # batch
H = 512         # hidden
NC_CLS = 4096   # classes per branch
NQ = 4          # class-quarters per branch
QCLS = NC_CLS // NQ
KC = H // 128   # contraction chunks

_cache = {}


def _build_shards(co_group_of, cl_group_of):
    """Per (branch, quarter): list of slots (group, cls_start, width<=64)."""
    shards = []
    for go in (co_group_of, cl_group_of):
        go = np.asarray(go).astype(np.int64)
        for q in range(NQ):
            c0, c1 = q * QCLS, (q + 1) * QCLS
            slots = []
            i = c0
            while i < c1:
                g = go[i]
                j = i
                while j < c1 and go[j] == g:
                    j += 1
                for s in range(i, j, 64):
                    slots.append((int(g), s, min(64, j - s)))
                i = j
            shards.append(slots)
    return shards


def _program(S, dt=mybir.dt.bfloat16):
    """Build the uniform SPMD Bass program for S slots per core."""
    nc = bacc.Bacc("TRN2", target_bir_lowering=False, debug=False, num_devices=8)
    xt_d = nc.dram_tensor("xt", [128, S * KC * 64], dt, kind="ExternalInput")
    wt_d = nc.dram_tensor("wt", [128, KC, S * 64], dt, kind="ExternalInput")
    bz_d = nc.dram_tensor("bz", [1, S * 64], dt, kind="ExternalInput")
    nhalf = ((S * 64 + 511) // 512 + 1) // 2
    o_d = nc.dram_tensor("o", [128, 512 * nhalf], mybir.dt.float32, kind="ExternalOutput")

    ntiles = (S * 64 + 511) // 512

    with tile.TileContext(nc) as tc:
        with (
            tc.tile_pool(name="xp", bufs=4 * ntiles) as xp,
            tc.tile_pool(name="wp", bufs=ntiles * KC) as wp,
            tc.tile_pool(name="cp", bufs=1) as cp,
            tc.tile_pool(name="op", bufs=ntiles) as op,
            tc.tile_pool(name="ps", bufs=min(ntiles, 8), space="PSUM") as ps,
        ):
            ones = cp.tile([1, 64], dt)
            nc.gpsimd.memset(ones[:], 1.0)
            bz = cp.tile([1, S * 64], dt)
            nc.scalar.dma_start(bz[:], bz_d[:])

            ohs = []
            for t in range(ntiles):
                s_lo = t * 8
                s_hi = min(S, s_lo + 8)
                nsl = s_hi - s_lo
                tw = nsl * 64

                xt = xp.tile([128, nsl * KC * 64], dt)
                nc.sync.dma_start(xt[:], xt_d[:, s_lo * KC * 64 : s_hi * KC * 64])
                wt = wp.tile([128, KC, tw], dt)
                nc.scalar.dma_start(wt[:], wt_d[:, :, s_lo * 64 : s_hi * 64])

                acc = ps.tile([64, 512], mybir.dt.float32)
                for sl in range(nsl):
                    for k in range(KC):
                        nc.tensor.matmul(
                            acc[0:64, sl * 64 : (sl + 1) * 64],
                            xt[:, (sl * KC + k) * 64 : (sl * KC + k + 1) * 64],
                            wt[:, k, sl * 64 : (sl + 1) * 64],
                            start=(k == 0),
                            stop=False,
                        )
                    nc.tensor.matmul(
                        acc[0:64, sl * 64 : (sl + 1) * 64],
                        ones[0:1, 0:64],
                        bz[0:1, (s_lo + sl) * 64 : (s_lo + sl + 1) * 64],
                        start=False,
                        stop=True,
                    )

                if t % 2 == 0:
                    oh = op.tile([128, 512], mybir.dt.float32)
                    ohs.append(oh)
                oh = ohs[t // 2]
                r0 = 64 * (t % 2)
                nc.vector.tensor_copy(oh[r0 : r0 + 64, 0:tw], acc[0:64, 0:tw])
                if t % 2 == 1 or t == ntiles - 1:
                    h = t // 2
                    eng = nc.sync if h % 2 == 0 else nc.scalar
                    eng.dma_start(o_d[:, h * 512 : (h + 1) * 512], oh[:])

    nc.compile()
    return nc


def _host_prep(x, W, bias, slots, S, goff):
    """Build xt/wt/bz arrays for one core."""
    nsl = len(slots)
    groups = np.array([g for g, _, _ in slots], np.int64)
    # xt: [128, S*KC*64]; col = s*(KC*64) + k*64 + b
    xg = x[:, goff + groups, :]                      # [B, nsl, H]
    xt = np.zeros((128, S * KC * 64), ml_dtypes.bfloat16)
    xt[:, : nsl * KC * 64] = (
        xg.reshape(B, nsl, KC, 128).transpose(3, 1, 2, 0).reshape(128, nsl * KC * 64)
    )
    # wt: [128, KC*S*64]; col = k*(S*64) + s*64 + j
    Wp = np.zeros((S * 64, H), ml_dtypes.bfloat16)
    bz = np.zeros((1, S * 64), ml_dtypes.bfloat16)
    for s, (g, cst, wdt) in enumerate(slots):
        Wp[s * 64 : s * 64 + wdt] = W[cst : cst + wdt]
        bz[0, s * 64 : s * 64 + wdt] = bias[cst : cst + wdt]
    wt = Wp.reshape(S * 64, KC, 128).transpose(2, 1, 0).reshape(128, KC * S * 64)
    return {"xt": xt, "wt": np.ascontiguousarray(wt).reshape(128, KC, S * 64), "bz": bz}


def kernel(x, co_W, cl_W, co_b, cl_b, co_group_of, cl_group_of, co_index,
           cl_index, group_len, _iters=1, _return_raw=False):
    x = np.asarray(x, np.float32)
    G = int(group_len)
    shards = _build_shards(co_group_of, cl_group_of)
    S = max(len(s) for s in shards)

    key = ("v5bf16", S)
    if key not in _cache:
        _cache[key] = _program(S)
    nc = _cache[key]

    Ws = (np.asarray(co_W, np.float32)[0], np.asarray(cl_W, np.float32)[0])
    bs = (np.asarray(co_b, np.float32)[0], np.asarray(cl_b, np.float32)[0])
    in_maps = []
    for k in range(8):
        bi, q = divmod(k, NQ)
        in_maps.append(_host_prep(x, Ws[bi], bs[bi], shards[k], S, bi * G))

    res = run_bass_kernel_spmd(nc, in_maps, list(range(8)))

    outs = []
    for bi, index in ((0, co_index), (1, cl_index)):
        full = np.empty((B, NC_CLS), np.float32)
        for q in range(NQ):
            slots = shards[bi * NQ + q]
            src = np.empty(QCLS, np.int64)
            for s, (g, cst, wdt) in enumerate(slots):
                src[cst - q * QCLS : cst - q * QCLS + wdt] = np.arange(
                    s * 64, s * 64 + wdt
                )
            oarr = res.results[bi * NQ + q]["o"]
            ntiles = (S * 64 + 511) // 512
            flat = np.empty((B, S * 64), np.float32)
            for t in range(ntiles):
                s_lo, s_hi = t * 8, min(S, t * 8 + 8)
                tw = (s_hi - s_lo) * 64
                r0 = 64 * (t % 2)
                flat[:, s_lo * 64 : s_lo * 64 + tw] = oarr[
                    r0 : r0 + 64, (t // 2) * 512 : (t // 2) * 512 + tw
                ]
            full[:, q * QCLS : (q + 1) * QCLS] = flat[:, src]
        outs.append(full[:, np.asarray(index).astype(np.int64)])
    return outs[0], outs[1]



# revision 2
# speedup vs baseline: 1.3959x; 1.3959x over previous
"""GroupWiseLinear Trainium2 kernel.

out[b, c] = dot(W[0, c, :], x[b, group_of[c], :]) + bias[0, c], then a final
class-permutation gather, for two independent branches (co / cl).

Sharding: the 128 ragged group-segments (64 per branch) are chopped into
pieces of <= T classes and LPT-assigned across all 8 cores (cores freely mix
branches; the host unshard step composes the final permutation, so class
placement is arbitrary).  Every core runs the SAME program over S slots whose
widths come from a shared profile = elementwise max of each core's sorted
piece widths (rounded up to 16), so the instruction stream is SPMD-uniform
while W columns are only padded by the profile slack.

Device layout per core (bf16):
  - xw: [128, XWCOLS]  chunked slabs, each = per-slot x^T stationaries
        ([128, 64] per (slot, k-chunk)) followed by the chunk's W^T columns
        ([128, KC, cols]).  One DMA per chunk.
  - bz: [1, totW]      bias in slot layout (zero-padded)
  - o:  [64, totW]     output (batch-major), bf16

Per slot: 4 accumulating matmuls (x stationary [128,64], W moving [128,w])
into a PSUM bank region, plus a rank-1 ones-matmul adding the bias.  PSUM
banks are packed first-fit with 512-col capacity; each full bank is converted
f32->bf16 by the DVE into the output buffer, which is DMA'd out in two pieces.
"""

import ml_dtypes
import numpy as np

import concourse.bacc as bacc
import concourse.tile as tile
from concourse import mybir
from concourse.bass_utils import run_bass_kernel_spmd

B = 64          # batch
H = 512         # hidden
G = 64          # groups per branch
KC = H // 128   # contraction chunks
NCORES = 8
CAP = 1024      # class columns per core (2 * 4096 / 8)
T = 256         # max piece width (chop granularity)
GRAN = 16       # slot width granularity
PSUM_COLS = 512
CHUNK_BYTES = 480 * 1024  # target slab size per DMA

_cache = {}


def _segments(go):
    """Runs of equal group id -> list of (group, class_start, length)."""
    go = np.asarray(go).astype(np.int64)
    segs = []
    n = len(go)
    i = 0
    while i < n:
        g = int(go[i])
        j = i
        while j < n and go[j] == g:
            j += 1
        segs.append((g, i, j - i))
        i = j
    return segs


def _plan(co_group_of, cl_group_of):
    """Chop segments, LPT-assign pieces to cores, build the shared profile."""
    pieces = []
    for b, go in ((0, co_group_of), (1, cl_group_of)):
        for g, st, L in _segments(go):
            off = 0
            while L > 0:
                w = min(T, L)
                pieces.append((b, g, st + off, w))
                off += w
                L -= w
    pieces.sort(key=lambda p: (-p[3], p[0], p[2]))
    loads = [0] * NCORES
    assign = [[] for _ in range(NCORES)]
    for p in pieces:
        c = min(range(NCORES), key=lambda k: (loads[k] + p[3] > CAP, loads[k], k))
        w = p[3]
        if loads[c] + w > CAP:
            room = CAP - loads[c]
            if room > 0:
                assign[c].append((p[0], p[1], p[2], room))
                loads[c] += room
                p = (p[0], p[1], p[2] + room, w - room)
            c = min(range(NCORES), key=lambda k: (loads[k], k))
        assign[c].append(p)
        loads[c] += p[3]
    for a in assign:
        a.sort(key=lambda p: (-p[3], p[0], p[2]))
    S = max(len(a) for a in assign)
    prof = []
    for i in range(S):
        m = max((a[i][3] if i < len(a) else 0) for a in assign)
        prof.append(int(-(-m // GRAN) * GRAN))
    return assign, prof


def _layout(prof):
    """PSUM bank packing + DMA chunking + column offsets, all profile-only."""
    S = len(prof)
    goff = [0]
    for w in prof:
        goff.append(goff[-1] + w)
    totW = goff[-1]

    # psum banks: sequential first-fit, 512-col capacity
    slot_bank = []
    banks = []  # (first_slot, used_cols, base_col)
    cur_used = 0
    cur_base = 0
    cur_first = 0
    for j, w in enumerate(prof):
        if cur_used + w > PSUM_COLS:
            banks.append((cur_first, cur_used, cur_base))
            cur_first = j
            cur_base = goff[j]
            cur_used = 0
        slot_bank.append(len(banks))
        cur_used += w
    banks.append((cur_first, cur_used, cur_base))

    # DMA chunks over slots: slab bytes = nsl*64KB (x) + cols*1KB (W)
    chunks = []  # (slot_lo, slot_hi)
    lo = 0
    acc = 0
    for j in range(S):
        acc += 64 * 1024 + prof[j] * 1024
        if acc >= CHUNK_BYTES or j == S - 1:
            chunks.append((lo, j + 1))
            lo = j + 1
            acc = 0
    return goff, totW, slot_bank, banks, chunks


def _program(prof, dt=mybir.dt.bfloat16):
    S = len(prof)
    goff, totW, slot_bank, banks, chunks = _layout(prof)
    nxw = sum(
        (hi - lo) * KC * 64 + KC * (goff[hi] - goff[lo]) for lo, hi in chunks
    )
    nc = bacc.Bacc("TRN2", target_bir_lowering=False, debug=False, num_devices=8)
    xw_d = nc.dram_tensor("xw", [128, nxw], dt, kind="ExternalInput")
    bz_d = nc.dram_tensor("bz", [1, totW], dt, kind="ExternalInput")
    o_d = nc.dram_tensor("o", [64, totW], dt, kind="ExternalOutput")

    with tile.TileContext(nc) as tc:
        with (
            tc.tile_pool(name="sb", bufs=1) as sb,
            tc.tile_pool(name="ps", bufs=1, space="PSUM") as ps,
        ):
            ones = sb.tile([1, 64], dt, tag="ones")
            nc.gpsimd.memset(ones[:], 1.0)
            bz = sb.tile([1, totW], dt, tag="bz")
            nc.scalar.dma_start(bz[:], bz_d[:])

            xw_tiles = []
            dbase = 0
            for ci, (lo, hi) in enumerate(chunks):
                cols = goff[hi] - goff[lo]
                ccols = (hi - lo) * KC * 64 + KC * cols
                xw = sb.tile([128, ccols], dt, tag=f"xw{ci}", name=f"xw{ci}")
                eng = nc.sync if ci % 2 == 0 else nc.scalar
                eng.dma_start(xw[:], xw_d[:, dbase : dbase + ccols])
                xw_tiles.append((xw, lo, hi, (hi - lo) * KC * 64, cols))
                dbase += ccols

            pbanks = [
                ps.tile([64, PSUM_COLS], mybir.dt.float32, tag=f"pb{i}", name=f"pb{i}")
                for i in range(len(banks))
            ]
            ob = sb.tile([64, totW], dt, tag="ob")

            out_splits = []
            for j in range(S):
                ci = next(i for i, (_, lo, hi, _, _) in enumerate(xw_tiles) if lo <= j < hi)
                xw, lo, hi, woff, cols = xw_tiles[ci]
                w = prof[j]
                bi = slot_bank[j]
                bfirst, bused, bbase = banks[bi]
                po = goff[j] - bbase
                acc = pbanks[bi]
                for k in range(KC):
                    nc.tensor.matmul(
                        acc[0:64, po : po + w],
                        xw[:, ((j - lo) * KC + k) * 64 : ((j - lo) * KC + k + 1) * 64],
                        xw[:, woff + k * cols + (goff[j] - goff[lo]) : woff + k * cols + (goff[j] - goff[lo]) + w],
                        start=(k == 0),
                        stop=False,
                    )
                nc.tensor.matmul(
                    acc[0:64, po : po + w],
                    ones[0:1, 0:64],
                    bz[0:1, goff[j] : goff[j] + w],
                    start=False,
                    stop=True,
                )
                # close out the bank when its last slot is done
                if j == S - 1 or slot_bank[j + 1] != bi:
                    nc.vector.tensor_copy(ob[0:64, bbase : bbase + bused], acc[0:64, 0:bused])
                    if bi == len(banks) - 2 or bi == len(banks) - 1:
                        out_splits.append(bbase + bused)

            # two output DMAs: everything up to the second-to-last bank, then the rest
            if len(banks) == 1:
                nc.sync.dma_start(o_d[:, 0:totW], ob[0:64, 0:totW])
            else:
                split = banks[-1][2]
                nc.sync.dma_start(o_d[:, 0:split], ob[0:64, 0:split])
                nc.scalar.dma_start(o_d[:, split:totW], ob[0:64, split:totW])

    nc.compile()
    return nc


def _host_prep(x, Ws, bs, pieces, prof):
    """Build xw/bz for one core.  pieces: list of (branch, group, cls0, w)."""
    S = len(prof)
    goff, totW, slot_bank, banks, chunks = _layout(prof)
    nxw = sum((hi - lo) * KC * 64 + KC * (goff[hi] - goff[lo]) for lo, hi in chunks)
    xw = np.zeros((128, nxw), ml_dtypes.bfloat16)
    bz = np.zeros((1, totW), ml_dtypes.bfloat16)
    dbase = 0
    for lo, hi in chunks:
        cols = goff[hi] - goff[lo]
        woff = dbase + (hi - lo) * KC * 64
        for j in range(lo, min(hi, len(pieces))):
            b, g, cls0, wr = pieces[j]
            # stationary x^T for this slot: [128, KC*64]
            xs = x[:, b * G + g, :].reshape(B, KC, 128).transpose(2, 1, 0).reshape(128, KC * 64)
            xw[:, dbase + (j - lo) * KC * 64 : dbase + (j - lo + 1) * KC * 64] = xs
            # W^T columns: [128, KC, wr] at local offset
            wseg = Ws[b][cls0 : cls0 + wr, :].reshape(wr, KC, 128).transpose(2, 1, 0)
            loc = goff[j] - goff[lo]
            for k in range(KC):
                xw[:, woff + k * cols + loc : woff + k * cols + loc + wr] = wseg[:, k, :]
            bz[0, goff[j] : goff[j] + wr] = bs[b][cls0 : cls0 + wr]
        dbase += (hi - lo) * KC * 64 + KC * cols
    return {"xw": xw, "bz": bz}


def kernel(x, co_W, cl_W, co_b, cl_b, co_group_of, cl_group_of, co_index,
           cl_index, group_len, _return_raw=False):
    x = np.asarray(x, np.float32)
    assign, prof = _plan(co_group_of, cl_group_of)
    goff, totW, slot_bank, banks, chunks = _layout(prof)

    key = ("v2var", tuple(prof))
    if key not in _cache:
        _cache.clear()
        _cache[key] = _program(prof)
    nc = _cache[key]

    Ws = (np.asarray(co_W, np.float32)[0], np.asarray(cl_W, np.float32)[0])
    bs = (np.asarray(co_b, np.float32)[0], np.asarray(cl_b, np.float32)[0])
    in_maps = [_host_prep(x, Ws, bs, assign[c], prof) for c in range(NCORES)]

    res = run_bass_kernel_spmd(nc, in_maps, list(range(NCORES)))

    NC_CLS = len(np.asarray(co_group_of))
    fulls = [np.empty((B, NC_CLS), np.float32) for _ in range(2)]
    for c in range(NCORES):
        o = np.asarray(res.results[c]["o"], ml_dtypes.bfloat16).astype(np.float32)
        for j, (b, g, cls0, wr) in enumerate(assign[c]):
            fulls[b][:, cls0 : cls0 + wr] = o[:, goff[j] : goff[j] + wr]
    co_out = fulls[0][:, np.asarray(co_index).astype(np.int64)]
    cl_out = fulls[1][:, np.asarray(cl_index).astype(np.int64)]
    return co_out, cl_out
